# revision 33
# baseline (speedup 1.0000x reference)
"""Multi-head attention (B=4, S=2048, D=1024, H=16) on 8 trn2 NeuronCores.

Sharding: core c = 2*b + g handles batch b, head-group g (8 heads, 512 dims).
Q/K/V projections are column-sharded (Megatron), Wo row-sharded; the Wo
partial sums for the two head-groups of each batch are reduced host-side.

Causal path (build_program_v2) design:
  - Activations in "transposed" [feature, seq] layout; host pre-transposes
    and pre-casts streams + weights to bf16 (PE bf16 = f32r rate; DMA bytes
    halve; rel err ~3.5e-3 vs the 2e-2 gate).  Scores stay f32r, PSUM f32.
  - One batched rearranged-AP DMA per tensor (DMA on trn2 is dispatch-bound
    at ~625ns/descriptor-set, so 52 big DMAs beat 263 chunk DMAs).
  - Scores as S^T [k, q] tiles; softmax denominators via a ones-column
    appended to V (PV computes [O^T; rowsum]); per-q normalize via DVE
    reciprocal + gpsimd partition_broadcast (f32 — gpsimd bf16 compute is
    software-emulated and catastrophically slow; PSUM is gpsimd-inaccessible).
  - Software-pipelined emission: head-major attention units (one [128,1024]
    exp per unit), PV lagging LAG units behind S; next block's K/V/Q loads +
    projections and the previous block's out-projection are interleaved as
    PE filler inside the ACT-bound attention phase.  PSUM: 2x[128,1024]
    scores + 2x[128,512] proj + 2x[65,512] out = 16KB exactly.
  - Causal: upper-triangular blocks skipped; diagonal tiles trim S/exp/PV
    and mask-mul work to the masked band (never exp'ing or multiplying
    uninitialized PSUM/SBUF — junk*0 can be NaN on HW).
  - Last block: reversed head order + dl-reversed out-proj accumulation so
    the final normalize chain overlaps out-proj matmuls; stores ride two
    HWDGE queues at the tail.
"""

import ml_dtypes
import numpy as np

import concourse.mybir as mybir
import concourse.tile as tile
from concourse import bacc
from concourse.bass_utils import run_bass_kernel_spmd

F32 = mybir.dt.float32
F32R = mybir.dt.float32r
BF16 = mybir.dt.bfloat16
B, S, D, H, DK = 4, 2048, 1024, 16, 64
DL = 512  # dims per head-group (8 heads * 64)
NH = 8  # heads per core
NHP = 4  # head pairs per core
NDI = D // 128  # 8   d_model chunks
NSC = S // 128  # 16  seq chunks (k side)
NSB = S // 512  # 4   seq blocks (q side)
NDC = DL // 128  # 4  local-dim chunks
EXP_GRP = 2  # k-chunks per exp instruction
NEG = -1.0e9


def _load_cast(env, dst3, dram_ap, chunk, nchunks, base):
    """DMA chunks straight into the f32r dst (f32r is bit-identical to f32)."""
    nc = env["nc"]
    for i in range(nchunks):
        nc.sync.dma_start(
            out=dst3[:, i, :chunk], in_=dram_ap[i * 128 : (i + 1) * 128, :]
        )


def _emit_kproj(env, sb):
    nc = env["nc"]
    ssl = slice(sb * 512, (sb + 1) * 512)
    kt = env["stream"].tile([128, NDI, 512], F32R, tag="qkstream", name="kt")
    _load_cast(env, kt, env["kT_d"].ap()[:, ssl], 512, NDI, "ktraw")
    for dc in range(NDC):
        ps = env["pss"].tile([128, 512], F32, tag="s", name="ppk")
        for di in range(NDI):
            nc.tensor.matmul(
                ps[:],
                env["wk_r"][:, di, dc * 128 : (dc + 1) * 128],
                kt[:, di, :],
                start=(di == 0),
                stop=(di == NDI - 1),
            )
        nc.vector.tensor_scalar_add(
            env["Ksb"][:, dc, ssl], ps[:], env["bk_sb"][:, dc : dc + 1]
        )


def _emit_vproj(env, sc):
    nc = env["nc"]
    vtr = env["vstream"].tile([128, NDI, 128], F32R, tag="vtr", name="vtr")
    _load_cast(
        env, vtr, env["vT_d"].ap()[:, sc * 128 : (sc + 1) * 128], 128, NDI, "vtraw"
    )
    ps = env["pss"].tile([128, DL], F32, tag="s", name="ppv")
    for di in range(NDI):
        nc.tensor.matmul(
            ps[:],
            vtr[:, di, :],
            env["wv_r"][:, di, :],
            start=(di == 0),
            stop=(di == NDI - 1),
        )
    vt3 = env["vts"][sc][:].rearrange("p (h c) -> p h c", h=NH)
    nc.vector.tensor_add(
        vt3[:, :, 0:64],
        ps[:].rearrange("p (h c) -> p h c", h=NH),
        env["bv_sb"][:].rearrange("p (h c) -> p h c", h=NH),
    )
    nc.vector.tensor_copy(vt3[:, :, 64:65], env["ones_sb"][:].unsqueeze(2))


def _emit_s_exp(env, qb, hp, grp, mq_sb, skip_mask=False, skip_exp=False):
    nc = env["nc"]
    variant = env["variant"]
    pss, ep = env["pss"], env["ep"]
    Ksb, Qblk, mt_sb = env["Ksb"], env["Qblk"], env["mt_sb"]
    ng = len(grp)
    ps_s = {
        0: pss.tile([128, EXP_GRP * 512], F32, tag="s", name="pssA"),
        1: pss.tile([128, EXP_GRP * 512], F32, tag="s", name="pssB"),
    }
    for hb, (p0, p1) in enumerate(((0, 64), (64, 128))):
        for j, kc in enumerate(grp):
            nc.tensor.matmul(
                ps_s[hb][:, j * 512 : (j + 1) * 512],
                Ksb[p0:p1, hp, kc * 128 : (kc + 1) * 128],
                Qblk[p0:p1, hp, :],
                start=True,
                stop=True,
            )
    if variant == "general" and not skip_mask:
        for j, kc in enumerate(grp):
            psl = slice(j * 512, (j + 1) * 512)
            for hb in range(2):
                nc.vector.tensor_add(
                    ps_s[hb][:, psl], ps_s[hb][:, psl], mq_sb[:, kc, :]
                )
    if skip_exp:
        return None
    es = {}
    for hb in range(2):
        et = ep.tile([128, EXP_GRP * 512], F32R, tag="e", name=f"e{hb}")
        nc.scalar.activation(
            et[:, : ng * 512],
            ps_s[hb][:, : ng * 512],
            mybir.ActivationFunctionType.Exp,
            scale=1.0 / np.sqrt(DK),
        )
        es[hb] = et
    # causal: multiplicative zeroing post-exp on SBUF (off the ACT chain)
    if variant == "causal" and not skip_mask:
        for j, kc in enumerate(grp):
            if kc >= 4 * qb:
                psl = slice(j * 512, (j + 1) * 512)
                for hb in range(2):
                    nc.vector.tensor_mul(
                        es[hb][:, psl], es[hb][:, psl], mt_sb[:, kc - 4 * qb, :]
                    )
    return es


def _emit_pv(env, hp, grp, first, last, es, ps_o):
    nc = env["nc"]
    vts = env["vts"]
    for hb in range(2):
        for j, kc in enumerate(grp):
            h = 2 * hp + hb
            nc.tensor.matmul(
                ps_o[hb][:],
                vts[kc][:, h * 65 : (h + 1) * 65],
                es[hb][:, j * 512 : (j + 1) * 512],
                start=(kc == first),
                stop=(kc == last),
            )


def _emit_qblock(env, qb):
    nc = env["nc"]
    variant = env["variant"]
    skip = env["skip"]
    qsl = slice(qb * 512, (qb + 1) * 512)

    # Q projection for this q block
    qt = env["stream"].tile([128, NDI, 512], F32R, tag="qkstream", name="qt")
    _load_cast(env, qt, env["qT_d"].ap()[:, qsl], 512, NDI, "qtraw")
    Qblk = env["qbp"].tile([128, NDC, 512], F32R, tag="Qblk", name="Qblk")
    env["Qblk"] = Qblk
    for dc in range(NDC):
        ps = env["pss"].tile([128, 512], F32, tag="s", name="ppq")
        for di in range(NDI):
            nc.tensor.matmul(
                ps[:],
                env["wq_r"][:, di, dc * 128 : (dc + 1) * 128],
                qt[:, di, :],
                start=(di == 0),
                stop=(di == NDI - 1),
            )
        nc.vector.tensor_scalar_add(
            Qblk[:, dc, :], ps[:], env["bq_sb"][:, dc : dc + 1]
        )

    mq_sb = None
    if variant == "general":
        mq_sb = env["mqp"].tile([128, NSC, 512], F32, tag="mq", name="mq")
        nc.sync.dma_start(
            out=mq_sb[:],
            in_=env["mT_d"].ap()[:, qsl].rearrange("(c p) q -> p c q", p=128),
        )

    kept = env["kept_kcs"](qb)
    groups = [kept[i : i + EXP_GRP] for i in range(0, len(kept), EXP_GRP)]
    first, last = kept[0], kept[-1]
    skip_pv = "pv" in skip or "exp" in skip
    skip_mask = "nomask" in skip or "exp" in skip
    skip_exp = "exp" in skip
    Xblk = None
    if not ({"pv", "exp", "attn"} & set(skip)):
        Xblk = env["xbp"].tile([128, NDC, 512], F32R, tag="Xblk", name="Xblk")

    def _normalize(hp, ps_o):
        for hb, (p0, p1) in enumerate(((0, 64), (64, 128))):
            r = env["rp"].tile([1, 512], F32, tag="r", name=f"r{hb}")
            rb = env["rp"].tile([64, 512], F32, tag="rb", name=f"rb{hb}")
            nc.vector.reciprocal(r[:], ps_o[hb][64:65, :])
            nc.gpsimd.partition_broadcast(rb[:], r[0:1, :])
            nc.vector.tensor_mul(Xblk[p0:p1, hp, :], ps_o[hb][0:64, :], rb[:])

    if "attn" not in skip:
        units = [(hp, gi, grp) for hp in range(NHP)
                 for gi, grp in enumerate(groups)]
        ps_os = {}
        pend = None
        for hp, gi, grp in units:
            es = _emit_s_exp(env, qb, hp, grp, mq_sb,
                             skip_mask=skip_mask, skip_exp=skip_exp)
            if pend is not None and not skip_pv:
                phw, pgi, pgrp, pes = pend
                if pgi == 0:
                    ps_os[phw] = {
                        0: env["pso"].tile([65, 512], F32, tag="oA", name="psoA"),
                        1: env["pso"].tile([65, 512], F32, tag="oB", name="psoB"),
                    }
                _emit_pv(env, phw, pgrp, first, last, pes, ps_os[phw])
                if pgi == len(groups) - 1:
                    _normalize(phw, ps_os.pop(phw))
            pend = (hp, gi, grp, es)
        if pend is not None and not skip_pv:
            phw, pgi, pgrp, pes = pend
            if pgi == 0:
                ps_os[phw] = {
                    0: env["pso"].tile([65, 512], F32, tag="oA", name="psoA"),
                    1: env["pso"].tile([65, 512], F32, tag="oB", name="psoB"),
                }
            _emit_pv(env, phw, pgrp, first, last, pes, ps_os[phw])
            _normalize(phw, ps_os.pop(phw))

    # ---- out projection for this q block (stores ride the ACT queue) ----
    if {"out", "pv", "attn", "exp"} & set(skip):
        return
    for ec in range(NDI):
        ps = env["pss"].tile([128, 512], F32, tag="s", name="ppc")
        for dl in range(NDC):
            nc.tensor.matmul(
                ps[:],
                env["wo_r"][:, dl, ec * 128 : (ec + 1) * 128],
                Xblk[:, dl, :],
                start=(dl == 0),
                stop=(dl == NDC - 1),
            )
        ot = env["osp"].tile([128, 512], F32, tag="ot", name="ot")
        nc.scalar.copy(ot[:], ps[:])
        nc.scalar.dma_start(
            out=env["outT_d"].ap()[ec * 128 : (ec + 1) * 128, qsl], in_=ot[:]
        )



# ---------------------------------------------------------------------------
# v2 causal emitter: software-pipelined, prefetching, engine-balanced
# ---------------------------------------------------------------------------

LAG = 5  # units of S->PV pipeline distance


def build_program_v2(reps=1):
    """Causal MHA, restructured:
      - di-outer projections (2 PSUM accumulators) so compute starts after the
        first DMA chunk instead of the full 2MB stream.
      - next block's streams + projections emitted as PE filler interleaved
        into the (ACT-bound) attention phase of the current block.
      - head-major attention units (one [128,1024] exp per unit), PV lagging
        LAG units behind S; PSUM: 2x[128,1024] scores + 2x[128,512] proj +
        2x[65,512] outputs = 16KB exactly.
      - drains: K/Q-proj bias-adds on DVE, V-proj/out-proj drains + output
        stores on Pool (gpsimd), mask-muls round-robin DVE/Pool.
    """
    nc = bacc.Bacc("TRN2", target_bir_lowering=False, debug=False)

    qT_d = nc.dram_tensor("qT", [D, S], BF16, kind="ExternalInput")
    kT_d = nc.dram_tensor("kT", [D, S], BF16, kind="ExternalInput")
    vT_d = nc.dram_tensor("vT", [D, S], BF16, kind="ExternalInput")
    wq_d = nc.dram_tensor("wq", [D, DL], BF16, kind="ExternalInput")
    wk_d = nc.dram_tensor("wk", [D, DL], BF16, kind="ExternalInput")
    wv_d = nc.dram_tensor("wv", [D, DL], BF16, kind="ExternalInput")
    wo_d = nc.dram_tensor("wo", [DL, D], BF16, kind="ExternalInput")
    bq_d = nc.dram_tensor("bq", [128, NDC], F32, kind="ExternalInput")
    bk_d = nc.dram_tensor("bk", [128, NDC], F32, kind="ExternalInput")
    bv_d = nc.dram_tensor("bv", [128, DL], F32, kind="ExternalInput")
    mt_d = nc.dram_tensor("maskt", [4, 128, 512], BF16, kind="ExternalInput")
    outT_d = nc.dram_tensor("outT", [D, S], F32, kind="ExternalOutput")

    with tile.TileContext(nc) as tc:
        for _rep in range(reps):
            with (
                tc.tile_pool(name="persist", bufs=1) as pers,
                tc.tile_pool(name="wts", bufs=1) as wts,
                tc.tile_pool(name="vt", bufs=1) as vtp,
                tc.tile_pool(name="stream", bufs=2) as stream,
                tc.tile_pool(name="vstream", bufs=2) as vstream,
                tc.tile_pool(name="qblk", bufs=2) as qbp,
                tc.tile_pool(name="xblk", bufs=2) as xbp,
                tc.tile_pool(name="epool", bufs=6) as ep,
                tc.tile_pool(name="ostage", bufs=3) as osp,
                tc.tile_pool(name="psum", bufs=2, space="PSUM") as psp,
            ):
                # ---- constants ----
                bq_sb = pers.tile([128, NDC], F32, tag="bq", name="bq_sb")
                bk_sb = pers.tile([128, NDC], F32, tag="bk", name="bk_sb")
                bv_sb = pers.tile([128, DL], F32, tag="bv", name="bv_sb")
                ones_sb = pers.tile([128, NH], BF16, tag="ones", name="ones_sb")
                nc.sync.dma_start(out=bq_sb[:], in_=bq_d.ap())
                nc.sync.dma_start(out=bk_sb[:], in_=bk_d.ap())
                nc.sync.dma_start(out=bv_sb[:], in_=bv_d.ap())
                nc.any.memset(ones_sb[:], 1.0)

                # ---- persistent activations ----
                Ksb = pers.tile([128, NDC, S], F32R, tag="Ksb", name="Ksb")
                vts = [
                    vtp.tile([128, NH * 65], BF16, tag=f"vt{sc}", name=f"vt{sc}")
                    for sc in range(NSC)
                ]
                w_sbs = {}
                for wname in ("wk", "wv", "wq"):
                    w_sbs[wname] = wts.tile(
                        [128, NDI, DL], BF16, tag=wname, name=f"{wname}_sb"
                    )
                wo_sb = pers.tile([128, NDC, D], BF16, tag="wo", name="wo_sb")
                mt_sb = pers.tile([128, 4, 512], BF16, tag="mt", name="mt_sb")

                # ---------------- load emitters (SP queue) ----------------
                def load_w(wname, wd):
                    nc.sync.dma_start(
                        out=w_sbs[wname][:],
                        in_=wd.ap().rearrange("(c p) f -> p c f", p=128),
                    )

                def load_kt(sb):
                    ssl = slice(sb * 512, (sb + 1) * 512)
                    kt = stream.tile(
                        [128, NDI, 512], BF16, tag="kq", name=f"kt{sb}"
                    )
                    nc.sync.dma_start(
                        out=kt[:],
                        in_=kT_d.ap()[:, ssl].rearrange("(c p) q -> p c q", p=128),
                    )
                    return kt

                def load_qt(sb):
                    ssl = slice(sb * 512, (sb + 1) * 512)
                    qt = stream.tile(
                        [128, NDI, 512], BF16, tag="kq", name=f"qt{sb}"
                    )
                    nc.sync.dma_start(
                        out=qt[:],
                        in_=qT_d.ap()[:, ssl].rearrange("(c p) q -> p c q", p=128),
                    )
                    return qt

                def load_vblk(sb):
                    """One [128, NDI, 512] tile covering the block's 4 sc."""
                    ssl = slice(sb * 512, (sb + 1) * 512)
                    vtr = vstream.tile(
                        [128, NDI, 512], BF16, tag="vtr", name=f"vtrb{sb}"
                    )
                    nc.sync.dma_start(
                        out=vtr[:],
                        in_=vT_d.ap()[:, ssl].rearrange("(c p) q -> p c q", p=128),
                    )
                    return vtr

                # ------------- projection compute (chunk generators) -------
                def kq_proj_chunks(w_sb, xt, drain, pairs=(0, 1)):
                    """di-outer, dc-pair accumulation. drain(dcpair, psA, psB)"""
                    for pair in pairs:
                        psA = psp.tile([128, 512], F32, tag="p", name="ppA")
                        psB = psp.tile([128, 512], F32, tag="p", name="ppB")
                        for di in range(NDI):
                            def chunk(pair=pair, di=di, psA=psA, psB=psB):
                                for q, ps in ((0, psA), (1, psB)):
                                    dc = 2 * pair + q
                                    nc.tensor.matmul(
                                        ps[:],
                                        w_sb[:, di, dc * 128 : (dc + 1) * 128],
                                        xt[:, di, :],
                                        start=(di == 0),
                                        stop=(di == NDI - 1),
                                    )
                                if di == NDI - 1:
                                    drain(pair, psA, psB)
                            yield chunk

                _rot = [0]

                def drain_eng():
                    # PSUM readers must be DVE or ACT (gpsimd is SBUF-only)
                    return nc.vector

                def tail_drain_eng():
                    _rot[0] += 1
                    return nc.vector if _rot[0] % 2 else nc.scalar

                def kproj_drain(sb, pair, psA, psB):
                    ssl = slice(sb * 512, (sb + 1) * 512)
                    for q, ps in ((0, psA), (1, psB)):
                        dc = 2 * pair + q
                        drain_eng().tensor_scalar_add(
                            Ksb[:, dc, ssl], ps[:], bk_sb[:, dc : dc + 1]
                        )

                def qproj_drain(Qblk, pair, psA, psB):
                    for q, ps in ((0, psA), (1, psB)):
                        dc = 2 * pair + q
                        drain_eng().tensor_scalar_add(
                            Qblk[:, dc, :], ps[:], bq_sb[:, dc : dc + 1]
                        )

                def vproj_chunks(sc, vtr):
                    scl = slice((sc % 4) * 128, (sc % 4 + 1) * 128)
                    ps = psp.tile([128, DL], F32, tag="p", name="ppv")
                    for half in range(2):
                        def chunk(half=half, ps=ps):
                            for di in range(4 * half, 4 * half + 4):
                                nc.tensor.matmul(
                                    ps[:],
                                    vtr[:, di, scl],
                                    w_sbs["wv"][:, di, :],
                                    start=(di == 0),
                                    stop=(di == NDI - 1),
                                )
                            if half == 1:
                                vt3 = vts[sc][:].rearrange(
                                    "p (h c) -> p h c", h=NH
                                )
                                drain_eng().tensor_add(
                                    vt3[:, :, 0:64],
                                    ps[:].rearrange("p (h c) -> p h c", h=NH),
                                    bv_sb[:].rearrange("p (h c) -> p h c", h=NH),
                                )
                                nc.vector.tensor_copy(
                                    vt3[:, :, 64:65], ones_sb[:].unsqueeze(2)
                                )
                        yield chunk

                def block_filler(sbn, n_v=4, defer_k0=False):
                    """Loads + projections for block sbn, as a chunk list.
                    n_v < 4 defers the last V chunks to the next block;
                    defer_k0 defers kproj dc-pair 0 (needed by late heads)."""
                    chunks = []
                    deferred = []
                    kt = {}
                    vtrs = {}
                    qt = {}
                    chunks.append(lambda: kt.__setitem__(0, load_kt(sbn)))

                    def loadv():
                        vb = load_vblk(sbn)
                        for i in range(4):
                            vtrs[i] = vb

                    chunks.append(loadv)
                    chunks.append(lambda: qt.__setitem__(0, load_qt(sbn)))

                    def kq_gen():
                        kg = kq_proj_chunks(
                            w_sbs["wk"], kt[0],
                            lambda pr, a, b: kproj_drain(sbn, pr, a, b),
                            pairs=((1, 0) if defer_k0 else (0, 1)),
                        )
                        for _ in range(NDI):  # first pair only
                            yield next(kg)
                        if defer_k0:
                            deferred.extend([lambda g=kg: next(g)()] * NDI)
                        else:
                            for c in kg:
                                yield c
                        for i in range(n_v):
                            for c in vproj_chunks(4 * sbn + i, vtrs[i]):
                                yield c
                        Qblk = qbp.tile(
                            [128, NDC, 512], F32R, tag="Qblk", name=f"Qblk{sbn}"
                        )
                        qt[1] = Qblk
                        for c in kq_proj_chunks(
                            w_sbs["wq"], qt[0],
                            lambda pr, a, b: qproj_drain(Qblk, pr, a, b),
                        ):
                            yield c

                    gen = kq_gen()

                    def drive():
                        try:
                            next(gen)()
                        except StopIteration:
                            pass

                    n_drive = (NDI if defer_k0 else 2 * NDI) + 2 * n_v + 2 * NDI
                    for _ in range(n_drive):
                        chunks.append(drive)
                    return chunks, qt, vtrs, deferred

                def vlate_chunks(sbn, vtrs):
                    """Deferred vproj chunks for sc 2,3 of block sbn."""
                    def gen():
                        for i in (2, 3):
                            for c in vproj_chunks(4 * sbn + i, vtrs[i]):
                                yield c

                    g = gen()

                    def drive():
                        try:
                            next(g)()
                        except StopIteration:
                            pass

                    return [drive] * 4

                # ---------------- attention ----------------
                def attention(qb, Qblk, filler, Xblk, front_n=0):
                    kept = list(range(4 * qb + 4))
                    groups = [
                        kept[i : i + EXP_GRP]
                        for i in range(0, len(kept), EXP_GRP)
                    ]
                    first, last = kept[0], kept[-1]
                    heads = range(NH - 1, -1, -1) if qb == NSB - 1 else range(NH)
                    units = [
                        (h, gi, grp)
                        for h in heads
                        for gi, grp in enumerate(groups)
                    ]
                    nU = len(units)
                    nF = len(filler)
                    ps_os = {}
                    pend = []
                    fill_done = 0
                    mask_rr = [0]

                    def emit_unit(h, gi, grp):
                        hp, p0 = h // 2, (h % 2) * 64
                        ng = len(grp)
                        diag = grp[0] >= 4 * qb
                        j0 = grp[0] - 4 * qb if diag else 0
                        c0g = 128 * j0 if diag else 0
                        ps = psp.tile(
                            [128, EXP_GRP * 512], F32, tag="s", name="pss"
                        )
                        for j, kc in enumerate(grp):
                            nc.tensor.matmul(
                                ps[:, j * 512 : (j + 1) * 512],
                                Ksb[p0 : p0 + 64, hp, kc * 128 : (kc + 1) * 128],
                                Qblk[p0 : p0 + 64, hp, :],
                                start=True,
                                stop=True,
                            )
                        et = ep.tile(
                            [128, EXP_GRP * 512], BF16, tag="e", name="et"
                        )
                        exp = mybir.ActivationFunctionType.Exp
                        scl = 1.0 / np.sqrt(DK)
                        if diag and j0 == 2 and ng == 2:
                            # only cols PV reads: [256:512) of tile jo=2 and
                            # [384:512) of tile jo=3; stale cols are zeroed
                            # by the mask muls below or never read
                            nc.scalar.activation(
                                et[:, 256:512], ps[:, 256:512], exp, scale=scl
                            )
                            nc.scalar.activation(
                                et[:, 896:1024], ps[:, 896:1024], exp, scale=scl
                            )
                        else:
                            nc.scalar.activation(
                                et[:, : ng * 512],
                                ps[:, : ng * 512],
                                exp,
                                scale=scl,
                            )
                        if diag:  # zero masked cols (masked band only)
                            for j, kc in enumerate(grp):
                                jo = kc - 4 * qb
                                base = j * 512
                                if j0 == 2 and jo == 2:
                                    lo, hi = 256, 384
                                elif j0 == 2 and jo == 3:
                                    # fully-masked prefix was never exp'd:
                                    # memset it; band [384:512) is exp'd
                                    nc.vector.memset(et[:, base : base + 384], 0.0)
                                    lo, hi = 384, 512
                                else:
                                    lo, hi = 0, min(512, 128 * jo + 128)
                                nc.vector.tensor_mul(
                                    et[:, base + lo : base + hi],
                                    et[:, base + lo : base + hi],
                                    mt_sb[:, jo, lo:hi],
                                )
                        return et

                    def emit_pv(h, gi, grp, et):
                        if gi == 0:
                            ps_os[h] = psp.tile(
                                [65, 512], F32, tag="o", name=f"pso{h}"
                            )
                        for j, kc in enumerate(grp):
                            # restrict to unmasked cols; first/last stay full
                            # so the accumulation group start/stop covers all
                            c0 = 128 * (kc - 4 * qb) if kc >= 4 * qb else 0
                            if kc == first or kc == last:
                                c0 = 0
                            nc.tensor.matmul(
                                ps_os[h][:, c0:],
                                vts[kc][:, h * 65 : (h + 1) * 65],
                                et[:, j * 512 + c0 : (j + 1) * 512],
                                start=(kc == first),
                                stop=(kc == last),
                            )
                        if gi == len(groups) - 1:
                            hp, p0 = h // 2, (h % 2) * 64
                            ps_o = ps_os.pop(h)
                            r = osp.tile([1, 512], F32, tag="r", bufs=2, name="r")
                            rb = osp.tile(
                                [64, 512], F32, tag="rb", bufs=2, name="rb"
                            )
                            nc.vector.reciprocal(r[:], ps_o[64:65, :])
                            nc.gpsimd.partition_broadcast(rb[:], r[0:1, :])
                            nc.vector.tensor_mul(
                                Xblk[p0 : p0 + 64, hp, :],
                                ps_o[0:64, :],
                                rb[:],
                            )

                    for idx, (h, gi, grp) in enumerate(units):
                        et = emit_unit(h, gi, grp)
                        pend.append((h, gi, grp, et))
                        front = min(front_n, 2 * (idx + 1))
                        rest = (idx + 1) * (nF - front_n) // nU
                        quota = min(nF, front + rest)
                        if qb < NSB - 1:
                            quota = min(
                                nF, max(quota, (idx + 1) * nF // nU, 2 * (idx + 1))
                            )
                        while fill_done < quota:
                            filler[fill_done]()
                            fill_done += 1
                        if len(pend) > LAG:
                            emit_pv(*pend.pop(0))
                    while fill_done < nF:
                        filler[fill_done]()
                        fill_done += 1
                    while pend:
                        emit_pv(*pend.pop(0))

                def outproj_chunks(qb, Xblk, tail=False):
                    qsl = slice(qb * 512, (qb + 1) * 512)
                    dls = [3, 2, 1, 0] if tail else [0, 1, 2, 3]
                    chunks = []
                    for ec in range(NDI):
                        def chunk(ec=ec):
                            ps = psp.tile([128, 512], F32, tag="p", name="ppc")
                            for i, dl in enumerate(dls):
                                nc.tensor.matmul(
                                    ps[:],
                                    wo_sb[:, dl, ec * 128 : (ec + 1) * 128],
                                    Xblk[:, dl, :],
                                    start=(i == 0),
                                    stop=(i == NDC - 1),
                                )
                            ot = osp.tile(
                                [128, 512], F32, tag="ot", bufs=6, name="ot"
                            )
                            if tail:
                                eng = tail_drain_eng()
                                if eng is nc.scalar:
                                    eng.copy(ot[:], ps[:])
                                else:
                                    eng.tensor_copy(ot[:], ps[:])
                            else:
                                nc.vector.tensor_copy(ot[:], ps[:])
                            dq = nc.scalar if (tail and ec % 2) else nc.sync
                            dq.dma_start(
                                out=outT_d.ap()[ec * 128 : (ec + 1) * 128, qsl],
                                in_=ot[:],
                            )
                        chunks.append(chunk)
                    return chunks

                # ---------------- program ----------------
                # wk/kt in di-quarters so kproj's first di-chunks start early
                kt0 = stream.tile([128, NDI, 512], BF16, tag="kq", name="kt0")
                for hh in range(4):
                    dsl = slice(hh * 256, (hh + 1) * 256)
                    csl = slice(hh * 2, (hh + 1) * 2)
                    nc.sync.dma_start(
                        out=w_sbs["wk"][:, csl, :],
                        in_=wk_d.ap()[dsl, :].rearrange("(c p) f -> p c f", p=128),
                    )
                    nc.sync.dma_start(
                        out=kt0[:, csl, :],
                        in_=kT_d.ap()[dsl, 0:512].rearrange(
                            "(c p) q -> p c q", p=128
                        ),
                    )
                load_w("wv", wv_d)
                vblk0 = load_vblk(0)
                load_w("wq", wq_d)
                qt0 = load_qt(0)
                nc.sync.dma_start(
                    out=mt_sb[:],
                    in_=mt_d.ap().rearrange("j p q -> p j q"),
                )
                nc.sync.dma_start(
                    out=wo_sb[:],
                    in_=wo_d.ap().rearrange("(c p) f -> p c f", p=128),
                )

                # block 0 projections, not interleaved
                for c in kq_proj_chunks(
                    w_sbs["wk"], kt0, lambda pr, a, b: kproj_drain(0, pr, a, b)
                ):
                    c()
                for scc in range(4):
                    for c in vproj_chunks(scc, vblk0):
                        c()
                Qblk0 = qbp.tile([128, NDC, 512], F32R, tag="Qblk", name="Qblk0")
                for c in kq_proj_chunks(
                    w_sbs["wq"], qt0, lambda pr, a, b: qproj_drain(Qblk0, pr, a, b)
                ):
                    c()

                Qcur = Qblk0
                prev_out = []
                vlate = []
                deferred_k = []
                for sb in range(NSB):
                    if sb + 1 < NSB:
                        last = sb + 1 == NSB - 1
                        filler, qtd, vtrs_d, dk = block_filler(
                            sb + 1, n_v=(2 if last else 4), defer_k0=last
                        )
                        next_vlate = vlate_chunks(sb + 1, vtrs_d) if last else []
                    else:
                        filler, qtd, dk = [], {}, []
                        next_vlate = []
                    Xblk = xbp.tile(
                        [128, NDC, 512], BF16, tag="Xblk", name=f"Xblk{sb}"
                    )
                    # loads first (SP queue), then outproj + deferred kproj
                    # spread through the attention span, then next-block projs
                    nload = 3 if filler else 0
                    front = vlate + deferred_k
                    F = front + filler[:nload] + prev_out + filler[nload:]
                    attention(sb, Qcur, F, Xblk, front_n=len(front) + nload)
                    if sb + 1 < NSB:
                        prev_out = outproj_chunks(sb, Xblk)
                        vlate = next_vlate
                        deferred_k = dk
                        Qcur = qtd[1]
                    else:
                        for c in outproj_chunks(sb, Xblk, tail=True):
                            c()
    nc.compile()
    return nc

def build_program(variant, reps=1, skip=()):
    """variant: 'causal' | 'ones' | 'general'; skip: timing-ablation flags."""
    assert variant in ("causal", "ones", "general")
    if variant == "causal" and not skip:
        return build_program_v2(reps)
    nc = bacc.Bacc("TRN2", target_bir_lowering=False, debug=False)

    qT_d = nc.dram_tensor("qT", [D, S], F32R, kind="ExternalInput")
    kT_d = nc.dram_tensor("kT", [D, S], F32R, kind="ExternalInput")
    vT_d = nc.dram_tensor("vT", [D, S], F32R, kind="ExternalInput")
    wq_d = nc.dram_tensor("wq", [D, DL], F32R, kind="ExternalInput")
    wk_d = nc.dram_tensor("wk", [D, DL], F32R, kind="ExternalInput")
    wv_d = nc.dram_tensor("wv", [D, DL], F32R, kind="ExternalInput")
    wo_d = nc.dram_tensor("wo", [DL, D], F32R, kind="ExternalInput")
    bq_d = nc.dram_tensor("bq", [128, NDC], F32, kind="ExternalInput")
    bk_d = nc.dram_tensor("bk", [128, NDC], F32, kind="ExternalInput")
    bv_d = nc.dram_tensor("bv", [128, DL], F32, kind="ExternalInput")
    mt_d = mT_d = None
    if variant == "causal":
        # multiplicative 1/0 tiles for the 4 diagonal offsets [j, k, q]
        mt_d = nc.dram_tensor("maskt", [4, 128, 512], BF16, kind="ExternalInput")
    elif variant == "general":
        # additive 0/-1e9, transposed [k, q]
        mT_d = nc.dram_tensor("maskT", [S, S], F32, kind="ExternalInput")
    outT_d = nc.dram_tensor("outT", [D, S], F32, kind="ExternalOutput")

    def kept_kcs(qb):
        return list(range(4 * qb + 4)) if variant == "causal" else list(range(NSC))

    with tile.TileContext(nc) as tc:
        for _rep in range(reps):
            with (
                tc.tile_pool(name="persist", bufs=1) as pers,
                tc.tile_pool(name="wts", bufs=1) as wts,
                tc.tile_pool(name="vt", bufs=1) as vtp,
                tc.tile_pool(name="stream", bufs=1) as stream,
                tc.tile_pool(name="vstream", bufs=1) as vstream,
                tc.tile_pool(name="qblk", bufs=1) as qbp,
                tc.tile_pool(name="xblk", bufs=1) as xbp,
                tc.tile_pool(name="epool", bufs=4) as ep,
                tc.tile_pool(name="rpool", bufs=1) as rp,
                tc.tile_pool(name="ostage", bufs=2) as osp,
                tc.tile_pool(name="mq", bufs=1) as mqp,
                tc.tile_pool(name="pss", bufs=3, space="PSUM") as pss,
                tc.tile_pool(name="pso", bufs=1, space="PSUM") as pso,
            ):
                env = dict(
                    nc=nc, variant=variant, skip=skip, kept_kcs=kept_kcs,
                    qT_d=qT_d, kT_d=kT_d, vT_d=vT_d, mT_d=mT_d, outT_d=outT_d,
                    stream=stream, vstream=vstream, qbp=qbp,
                    xbp=xbp, ep=ep, rp=rp, osp=osp, mqp=mqp, pss=pss, pso=pso,
                )
                # ---- constants ----
                bq_sb = pers.tile([128, NDC], F32, tag="bq", name="bq_sb")
                bk_sb = pers.tile([128, NDC], F32, tag="bk", name="bk_sb")
                bv_sb = pers.tile([128, DL], F32, tag="bv", name="bv_sb")
                ones_sb = pers.tile([128, NH], F32, tag="ones", name="ones_sb")
                nc.sync.dma_start(out=bq_sb[:], in_=bq_d.ap())
                nc.sync.dma_start(out=bk_sb[:], in_=bk_d.ap())
                nc.sync.dma_start(out=bv_sb[:], in_=bv_d.ap())
                nc.any.memset(ones_sb[:], 1.0)
                env.update(bq_sb=bq_sb, bk_sb=bk_sb, bv_sb=bv_sb, ones_sb=ones_sb)

                mt_sb = None
                if variant == "causal":
                    mt_sb = pers.tile([128, 4, 512], BF16, tag="mt", name="mt_sb")
                    for j in range(4):
                        nc.sync.dma_start(out=mt_sb[:, j, :], in_=mt_d.ap()[j])
                env["mt_sb"] = mt_sb

                for wname, wd in (("wk", wk_d), ("wv", wv_d), ("wq", wq_d)):
                    w_sb = wts.tile(
                        [128, NDI, DL], F32R, tag=wname, name=f"{wname}_sb"
                    )
                    _load_cast(env, w_sb, wd.ap(), DL, NDI, f"{wname}raw")
                    env[f"{wname}_r"] = w_sb[:]

                wo_sb = pers.tile([128, NDC, D], F32R, tag="wo", name="wo_sb")
                for dl in range(NDC):
                    for hh in range(2):
                        nc.sync.dma_start(
                            out=wo_sb[:, dl, hh * 512 : (hh + 1) * 512],
                            in_=wo_d.ap()[
                                dl * 128 : (dl + 1) * 128, hh * 512 : (hh + 1) * 512
                            ],
                        )
                env["wo_r"] = wo_sb[:]

                Ksb = pers.tile([128, NDC, S], F32R, tag="Ksb", name="Ksb")
                vts = [
                    vtp.tile([128, NH * 65], F32R, tag=f"vt{sc}", name=f"vt{sc}")
                    for sc in range(NSC)
                ]
                env.update(Ksb=Ksb, vts=vts)

                if variant == "causal":
                    # interleave K/V projection blocks with attention blocks
                    for sb in range(NSB):
                        if "qk" not in skip:
                            _emit_kproj(env, sb)
                        if "v" not in skip:
                            for sc in range(4 * sb, 4 * sb + 4):
                                _emit_vproj(env, sc)
                        _emit_qblock(env, sb)
                else:
                    if "qk" not in skip:
                        for sb in range(NSB):
                            _emit_kproj(env, sb)
                    if "v" not in skip:
                        for sc in range(NSC):
                            _emit_vproj(env, sc)
                    for qb in range(NSB):
                        _emit_qblock(env, qb)
    nc.compile()
    return nc


# ---------------------------------------------------------------------------
# host side
# ---------------------------------------------------------------------------

_NC_CACHE = {}


def _get_program(variant, reps=1):
    key = (variant, reps)
    if key not in _NC_CACHE:
        _NC_CACHE[key] = build_program(variant, reps)
    return _NC_CACHE[key]


def detect_variant(mask):
    m = np.asarray(mask)
    if (m != 0).all():
        return "ones"
    tril = np.tril(np.ones((S, S), np.int8))
    for b in range(m.shape[0]):
        mb = (m[b] != 0).astype(np.int8)
        if not np.array_equal(mb, tril):
            return "general"
    return "causal"


def make_causal_mask_tiles():
    j = np.arange(4)[:, None, None]
    k = np.arange(128)[None, :, None]
    q = np.arange(512)[None, None, :]
    # multiplicative: 1 keep, 0 drop (applied to exp'd scores)
    return (q >= k + 128 * j).astype(np.float32)


def build_in_maps(query, key, value, mask, Wq, bq, Wk, bk, Wv, bv, Wo, bo, variant):
    query = np.asarray(query, np.float32)
    key = np.asarray(key, np.float32)
    value = np.asarray(value, np.float32)
    Wq, Wk, Wv, Wo = (np.asarray(w, np.float32) for w in (Wq, Wk, Wv, Wo))
    bq, bk, bv = (np.asarray(x, np.float32) for x in (bq, bk, bv))

    if variant == "causal":
        mtiles = make_causal_mask_tiles()

    in_maps = []
    for c in range(8):
        b, g = c // 2, c % 2
        gs = slice(g * DL, (g + 1) * DL)
        sdt = ml_dtypes.bfloat16 if variant == "causal" else np.float32
        m = {
            "qT": np.ascontiguousarray(query[b].T).astype(sdt),
            "kT": np.ascontiguousarray(key[b].T).astype(sdt),
            "vT": np.ascontiguousarray(value[b].T).astype(sdt),
            "wq": np.ascontiguousarray(Wq[gs].T).astype(sdt),
            "wk": np.ascontiguousarray(Wk[gs].T).astype(sdt),
            "wv": np.ascontiguousarray(Wv[gs].T).astype(sdt),
            "wo": np.ascontiguousarray(Wo[:, gs].T).astype(sdt),
            "bq": np.ascontiguousarray(bq[gs].reshape(NDC, 128).T),
            "bk": np.ascontiguousarray(bk[gs].reshape(NDC, 128).T),
            "bv": np.ascontiguousarray(np.broadcast_to(bv[gs], (128, DL))),
        }
        if variant == "causal":
            m["maskt"] = mtiles.astype(ml_dtypes.bfloat16)
        elif variant == "general":
            m["maskT"] = np.ascontiguousarray(
                np.where(np.asarray(mask[b]) != 0, 0.0, NEG).astype(np.float32).T
            )
        in_maps.append(m)
    return in_maps


def assemble_output(results, bo):
    bo = np.asarray(bo, np.float32)
    out = np.empty((B, S, D), np.float32)
    for b in range(B):
        acc = results[2 * b]["outT"] + results[2 * b + 1]["outT"]
        out[b] = acc.T + bo
    return out


def kernel(query, key, value, mask, Wq, bq, Wk, bk, Wv, bv, Wo, bo):
    variant = detect_variant(np.asarray(mask))
    in_maps = build_in_maps(
        query, key, value, mask, Wq, bq, Wk, bk, Wv, bv, Wo, bo, variant
    )
    nc = _get_program(variant)
    res = run_bass_kernel_spmd(nc, in_maps, core_ids=list(range(8)))
    return assemble_output(res.results, bo)

